# revision 43
# baseline (speedup 1.0000x reference)
"""Trainium2 Bass kernel for a DoReFa-quantized ResNet BasicBlock (inference).

Reference computation (all fp32):
    out = qact(bn2(conv3x3(qact(bn1(conv3x3(x, qw(w1)))), qw(w2))) + x)
with qw = 4-bit DoReFa weight quant, qact = 4-bit activation quant,
x: (64, 128, 56, 56), convs 128->128 stride 1 pad 1.

Sharding: data-parallel over the batch dim, 8 images per NeuronCore on 8 cores.

Per-core kernel design:
  * NCHW with C=128 on SBUF partitions, flattened zero-padded image rows in
    the free dim; a 3x3 conv = shifted 128x128 matmuls accumulated in PSUM.
    Half-image granularity: 28 output rows per half, 4 PSUM chunks of 7 rows
    (conv1 4 banks + conv2 4 banks = all 8).
  * conv1 runs in fp16 (x15 = 15*x shipped as fp16 from the host — the
    11-bit significand doubles the rounding of the f32r original but stays
    ~2x under the error gate, halves input HBM traffic, and streams at
    ~171 ns/392-col matmul vs 184 for f32r), tap-outer over 4 chunk
    matmuls per weight load.
    Quantized weights are exact small integers (15*w_q odd in [-15,15]) and
    activations 15*a in {0..15} (exact in fp8e4m3) -> conv2 is bit-exact
    integer fp8: 3 DoubleRow matmuls for the (dy=-1,+1) tap pairs, a 4th DR
    pair (dy=0,dx=-1)+(dx=+1) against a 16B-aligned shifted duplicate of
    act1 (made by SBUF->SBUF DMA on the gpsimd SW-DGE queue), 1 plain fp8
    matmul for the center tap. Measured on this part DR runs the array at
    1.2 GHz (vs 2.4 for normal mode), so DR's win is taps-per-pass, not
    cycles; pass-outer over 4 chunks amortizes the DR weight load.
  * A post-Tile pass rewrites the legalized module: redundant InstLdweights
    (same stationary operand as the previous load) become same-name NoOps,
    and repeated self-loading f32r matmuls get ldweights=False (validated
    bit-exact on HW); a second pass hoists walrus-illegal waits (>1 per
    instruction, any on a non-self-loading matmul) onto those NoOps.
  * BN folds into ScalarE activations out of PSUM (bn1 with Relu = the lower
    clip of the DoReFa staircase); staircase = tensor_scalar (min,+2^23) and
    (-2^23[,*1/15]) on VectorE, bit-matching jnp.round. qact1's second pass
    re-layouts W58 rows into the W64 act1 pitch for the fp8 DR pair stride.
  * Emission: load(n+1) queued while image n computes; PE order
    A(n,0) B(n-1,0) A(n,1) B(n-1,1) keeps the PE busy across the act1
    handoff. Steady-state input+output DMAs ride the dedicated sync HWDGE
    queue (a DMA trigger on the scalar engine blocks it head-of-line until
    the queue drains, starving the BN affines); only the first two images
    split across sync+scalar for the fill, with conv1 weights wrapped
    around image 0's chunks. The ScalarE activation table is pre-warmed
    with dummy ops so its lazy 1.3us load lands in the DMA fill.
  * Last image: conv2-h0 takes the freed conv1 PSUM banks, h1 runs as two
    2-chunk sub-phases, the residual is injected in PSUM via a diag(15/s2)
    f32r matmul with Relu folded into the bn2 affine, and the end-of-kernel
    drain is 2 DVE ops per 14-row piece.

Measured (8 cores, NTFF profile): ~181-183 us HW exec (baseline 194.1),
rel L2 err ~1.06e-2 (~1.1% of outputs off by one 1/15 quantization step;
gate 2e-2). PE ~162 us active: conv1 576 matmuls x ~171 ns, conv2 320 x
~177 ns (fp8 DoubleRow runs the array at 1.2 GHz, so DR's win is
taps-per-pass), plus ~7 us fixed preamble, ~4.5 us first-image DMA fill,
~8 us elementwise tail + teardown.
"""

import os
import sys

import numpy as np

for _p in ("/opt/trn_rl_repo", "/opt/pypackages"):
    if _p not in sys.path and os.path.isdir(_p):
        sys.path.insert(0, _p)

import ml_dtypes  # noqa: E402

# ---------------------------------------------------------------- constants
B, C, H, W = 64, 128, 56, 56
N_CORES = 8
BPC = B // N_CORES          # images per core
WP = W + 2                  # conv1/v1 padded row length (58)
WP2 = 64                    # conv2/act1/x padded row length (58 used + 6 dead)
HPAD = H + 2                # padded rows (58)
IMG = WP * HPAD             # v1-layout padded image elems (3364)
IMG2 = WP2 * HPAD           # act1/x-layout padded image elems (3712)
BUF = IMG + 4               # v1 buffer
BUF2 = IMG2 + 4             # act1/v2 buffer
XBASE = 16                  # x row r starts at byte 32r+34; all 3 dx-shifted
BUFX = XBASE + IMG2         # 112B row fetches span exactly 4 SBUF lines
ACT_D = 3726                # shifted act1 copy offset; pair step D+2 %16==0
ABUF = ACT_D + BUF2         # act1 tile width (original + shifted copy)
XB = 1                      # x / v buffers: image base offset
AB = 2                      # act1 buffer: base offset
RPC = 7                     # padded rows per PSUM chunk
CPH = 4                     # chunks per half (28 rows)
ROWS_H = RPC * CPH          # 28
FREE1 = RPC * W             # 392 free elems per conv1 matmul
FREE2 = RPC * WP            # 406 free elems per conv2 matmul
MAGIC = float(2**23)        # fp32 round-to-nearest-even magic constant
EPS = 1e-5

# input DMA row chunks (unpadded row ranges) per half
DMA_CHUNKS = (((0, 15), (15, 29)), ((29, 43), (43, 56)))

_CACHE = {}


def _dedupe_ldweights(nc, mybir):
    """Drop PE weight reloads whose stationary operand is identical to the
    immediately preceding load.

    Tile legalization pairs every fp8 InstMatmult with its own InstLdweights;
    with the tap-outer loop structure here, runs of 4 consecutive matmuls
    share the same weights, so 3 of every 4 loads are redundant. Replace each
    redundant InstLdweights with a same-name NoOp carrying its sync_info
    (keeps dependency edges and semaphore behavior intact). Self-loading
    matmuls (f32/f32r, which legalize does not split) clobber the array, so
    they reset the tracked key."""
    n = 0
    for fn in nc.m.functions:
        for blk in fn.blocks:
            last_key = None
            out = []
            for ins in blk.instructions:
                if ins.engine != mybir.EngineType.PE:
                    out.append(ins)
                    continue
                op = ins.opcode
                if op == "Ldweights":
                    key = (str(ins.ins[0]), str(ins.perf_mode),
                           str(ins.is_transpose))
                    if key == last_key:
                        nop = mybir.InstNoOp(name=ins.name, engine=ins.engine)
                        nop.sync_info = ins.sync_info
                        out.append(nop)
                        n += 1
                        continue
                    last_key = key
                elif op == "Matmult":
                    if getattr(ins, "ldweights", None) is not False:
                        # self-loading matmul (f32/f32r): if it repeats the
                        # previous stationary operand, skip its in-matmul
                        # weight load (validated on HW for f32r)
                        key = (str(ins.ins[1]), str(ins.perf_mode),
                               str(ins.is_transpose))
                        if key == last_key:
                            ins.ldweights = False
                            n += 1
                        else:
                            last_key = key
                elif op not in ("NoOp", "EventSemaphore"):
                    last_key = None
                out.append(ins)
            blk.instructions = out
    return n


# ---------------------------------------------------------------- host math
def _quant_weight_int(w):
    """Return 15*quantize_weight(w, 4) which is an exact odd integer in
    [-15, 15], as float32. Mirrors reference elementwise fp32 ops; tanh is
    computed in f64 and rounded (closest to any correctly-rounded f32 tanh)."""
    wt = np.tanh(w.astype(np.float64)).astype(np.float32)
    m = np.float32(np.abs(wt).max())
    wtn = wt / (np.float32(2.0) * m) + np.float32(0.5)      # [0, 1]
    q = np.round(wtn * np.float32(15.0)).astype(np.float32)  # {0..15}, half-even
    return np.float32(2.0) * q - np.float32(15.0)            # odd ints [-15,15]


def _bn_affine(gamma, beta, mean, var):
    """Per-channel (scale, bias) with bn(y) = scale*y + bias, in f64."""
    inv = 1.0 / np.sqrt(var.astype(np.float64) + EPS)
    s = gamma.astype(np.float64) * inv
    b = beta.astype(np.float64) - mean.astype(np.float64) * s
    return s, b


def _lhsT_taps(w_int):
    """[oc, ic, 3, 3] -> [ic, 9*oc] stationary-operand layout (tap-major)."""
    t = np.transpose(w_int, (2, 3, 1, 0)).reshape(9, C, C)   # [tap, ic, oc]
    return np.transpose(t, (1, 0, 2)).reshape(C, 9 * C)


# ---------------------------------------------------------------- bass build
def _fix_waits(nc, mybir):
    """Walrus encodes at most ONE sync wait per instruction, and none on a
    matmul marked non-self-loading by _dedupe_ldweights (its wait slot lives
    in the elided weight-load microcode).

    Hoist excess waits onto same-engine NoOps placed immediately before the
    instruction (the sequencer executes them in order, so semantics are
    unchanged)."""
    nid = 0
    for fn in nc.m.functions:
        for blk in fn.blocks:
            out = []
            changed = False
            for ins in blk.instructions:
                si = ins.sync_info
                noldw = (ins.opcode == "Matmult"
                         and getattr(ins, "ldweights", None) is False)
                if si is not None and (
                        len(si.on_wait) > 1 or (noldw and si.on_wait)):
                    waits = list(si.on_wait)
                    keep = [] if noldw else waits[-1:]
                    hoist = waits if noldw else waits[:-1]
                    for w in hoist:
                        nid += 1
                        nop = mybir.InstNoOp(name=f"I-wfix-{nid}",
                                             engine=ins.engine)
                        nop.sync_info = mybir.SyncInfo(on_wait=[w],
                                                       on_update=[])
                        out.append(nop)
                    ins.sync_info = mybir.SyncInfo(
                        on_wait=keep, on_update=list(si.on_update))
                    changed = True
                out.append(ins)
            if changed:
                blk.instructions = out


def _build_module():
    import concourse.bass as bass
    import concourse.mybir as mybir
    import concourse.tile as tile
    from contextlib import ExitStack

    f32 = mybir.dt.float32
    f16 = mybir.dt.float16
    f32r = mybir.dt.float32r
    f8 = mybir.dt.float8e4
    AF = mybir.ActivationFunctionType
    OP = mybir.AluOpType
    DR = mybir.MatmulPerfMode.DoubleRow

    nc = bass.Bass("TRN2", target_bir_lowering=False, debug=False,
                   num_devices=N_CORES)

    x_d = nc.dram_tensor("x15", [BPC, C, H, W], f16, kind="ExternalInput")
    # conv2 weights fp8: 3 DoubleRow pair blocks [2,128] (dy=-1/+1 per dx),
    # then the (dy=0,dx=-1)+(dx=+1) pair, then the center tap
    w2p_d = nc.dram_tensor("w2p", [C, 9 * C], f8, kind="ExternalInput")
    w1r_d = nc.dram_tensor("w1r", [C, 9 * C], f16, kind="ExternalInput")
    # columns: [sc1, bi1, sc2, bi2]
    bn_d = nc.dram_tensor("bnv", [C, 4], f32, kind="ExternalInput")
    # diag(15/s2) f32r: last-image residual injected via matmul into PSUM
    wdg_d = nc.dram_tensor("wdg", [C, C], f16, kind="ExternalInput")
    out_d = nc.dram_tensor("out", [BPC, C, H, W], f32, kind="ExternalOutput")

    inv15 = float(np.float32(1.0) / np.float32(15.0))
    HSPLIT = AB + (1 + ROWS_H) * WP2   # act1 byte split between halves

    with tile.TileContext(nc) as tc, ExitStack() as ctx:
        const = ctx.enter_context(tc.tile_pool(name="const", bufs=1))
        sb = ctx.enter_context(tc.tile_pool(name="sb", bufs=2))
        xp = ctx.enter_context(tc.tile_pool(name="xp", bufs=3))
        ps = ctx.enter_context(tc.tile_pool(name="ps", bufs=4, space="PSUM"))

        # conv1 weights: taps 0-2 up front on the scalar HWDGE queue (the
        # first matmul needs them); taps 3-8 deferred behind image 0's input
        w1r_sb = const.tile([C, 9 * C], f16)
        nc.scalar.dma_start(w1r_sb[:, 0:3 * C], w1r_d.ap()[:, 0:3 * C])
        wdg_sb = const.tile([C, C], f16)
        warm = const.tile([C, 128], f16)
        warmf = const.tile([C, 1], f32)
        nc.gpsimd.memset(warm[:], 0.0)
        nc.gpsimd.memset(warmf[:], 0.0)
        nc.scalar.activation(warmf[:], warmf[:], AF.Relu)
        nc.scalar.activation(warmf[:], warmf[:], AF.Identity)
        bn_sb = const.tile([C, 4], f32)
        w2p_sb = const.tile([C, 9 * C], f8)
        sc1_sb = bn_sb[:, 0:1]
        bi1_sb = bn_sb[:, 1:2]
        sc2_sb = bn_sb[:, 2:3]
        bi2_sb = bn_sb[:, 3:4]

        def wpair(p):
            return w2p_sb[:, p * 2 * C:(p + 1) * 2 * C].rearrange(
                "p (two m) -> p two m", two=2)

        def emit_load(n, h, tiles):
            """Input DMA + zero-padding for half h of image n."""
            if h == 0:
                x = xp.tile([C, BUFX], f16, tag="x", name=f"x_{n}")
                xs = sb.tile([C, H * W], f16, tag="xs", name=f"xs_{n}")
                tiles[n] = (x, xs)
            x, xs = tiles[n]
            xwr = x[:, XBASE:XBASE + IMG2].rearrange("p (h w) -> p h w", w=WP2)
            xr0 = x[:, XBASE:XBASE + IMG2].rearrange("p (h w) -> p h w", w=WP2)

            # zero pad borders for this half's rows (+ head/tail slack)
            # h0's dy=+1 taps read x row 29's pad cols, so zero rows 1..29
            # here and 30..56 in h1 (row 57 is covered by the tail memset)
            pr0, pr1 = (1, 1 + ROWS_H + 1) if h == 0 else (1 + ROWS_H + 1, 57)
            if h == 0:
                nc.gpsimd.memset(x[:, 0:XBASE + WP2 + 1], 0.0)   # slack+row0
            else:
                nc.gpsimd.memset(x[:, XBASE + (HPAD - 1) * WP2:BUFX], 0.0)
            nc.gpsimd.memset(xr0[:, pr0:pr1, 0], 0.0)            # left pad col
            nc.gpsimd.memset(xr0[:, pr0:pr1, 57], 0.0)           # right pad col

            # contiguous DMAs into staging (split across the two HWDGE
            # queues), then DVE pad-insert
            xsr = xs.rearrange("p (h w) -> p h w", w=W)
            xd_flat = x_d.ap()[n].rearrange("p h w -> p (h w)")
            for ci, (a, b) in enumerate(DMA_CHUNKS[h]):
                eng = nc.scalar if (ci == 1 and n <= 1) else nc.sync
                eng.dma_start(xs[:, a * W:b * W], xd_flat[:, a * W:b * W])
                nc.vector.tensor_copy(xwr[:, a + 1:b + 1, 1:57],
                                      xsr[:, a:b, :])

            if n == 0 and h == 0:
                # PE p-state pre-warm: dummy matmuls during the DMA fill so
                # the array clock is ramped when the real stream starts
                pw = ps.tile([C, 128], f32, tag="pb", bufs=CPH, name="pwarm")
                for _ in range(40):
                    nc.tensor.matmul(pw[:], lhsT=warm[:, 0:128],
                                     rhs=warm[:, 0:128], start=True, stop=True)
                for q in (1, 2):
                    nc.scalar.dma_start(w1r_sb[:, q * 3 * C:(q + 1) * 3 * C],
                                        w1r_d.ap()[:, q * 3 * C:(q + 1) * 3 * C])
                # deferred const loads on the gpsimd SW-DGE queue: off both
                # HWDGE queues' critical paths (bn first read ~6us in, w2p
                # at the first conv2)
                nc.gpsimd.dma_start(bn_sb[:], bn_d.ap())
                nc.gpsimd.dma_start(w2p_sb[:], w2p_d.ap())
                nc.gpsimd.dma_start(wdg_sb[:], wdg_d.ap())

        def emit_A(n, h, tiles):
            """conv1 + bn1 + qact for half h of image n."""
            x, xs = tiles[n]
            if h == 0:
                v1 = sb.tile([C, BUF], f32, tag="v1", name=f"v1_{n}")
                act1 = sb.tile([C, ABUF], f8, tag="act1", name=f"act1_{n}")
                tiles[(n, "a")] = (v1, act1)
            v1, act1 = tiles[(n, "a")]
            rr0, rr1 = 1 + ROWS_H * h, 1 + ROWS_H * (h + 1)

            # conv1, tap-outer: one weight load per tap, 4 chunk matmuls
            pa = [ps.tile([C, FREE1], f32, tag="pa", bufs=CPH,
                          name=f"pa_{n}_{h}_{c}") for c in range(CPH)]
            for t9 in range(9):
                dy, dx = t9 // 3 - 1, t9 % 3 - 1
                for c in range(CPH):
                    r0 = rr0 + RPC * c
                    off = XBASE + (r0 + dy) * WP2 + 1 + dx
                    mv = bass.AP(tensor=x.tensor, offset=off,
                                 ap=[[BUFX, C], [WP2, RPC], [1, W]])
                    nc.tensor.matmul(
                        pa[c][:], lhsT=w1r_sb[:, t9 * C:(t9 + 1) * C],
                        rhs=mv, start=(t9 == 0), stop=(t9 == 8))
            # bn1 affine out of PSUM; Relu = lower clip of the staircase
            for c in range(CPH):
                r0 = rr0 + RPC * c
                dst = v1[:, XB + r0 * WP:XB + (r0 + RPC) * WP].rearrange(
                    "p (h w) -> p h w", w=WP)[:, :, 1:57]
                nc.scalar.activation(dst,
                                     pa[c][:].rearrange(
                                         "p (h w) -> p h w", w=W),
                                     AF.Relu, bias=bi1_sb, scale=sc1_sb)

            # qact: min(.,15) then +M (one DVE pass), then -M into fp8 act1
            # (second pass also re-layouts W58 -> W64 rows)
            lo, hi = XB + rr0 * WP, XB + rr1 * WP
            nc.vector.tensor_scalar(v1[:, lo:hi], v1[:, lo:hi],
                                    15.0, MAGIC, op0=OP.min, op1=OP.add)
            v1r = v1[:, XB:XB + IMG].rearrange("p (h w) -> p h w", w=WP)
            ar = act1[:, AB:AB + IMG2].rearrange("p (h w) -> p h w", w=WP2)
            nc.vector.tensor_scalar(ar[:, rr0:rr1, 0:WP], v1r[:, rr0:rr1, 0:WP],
                                    MAGIC, None, op0=OP.subtract)
            if h == 0:
                nc.gpsimd.memset(act1[:, 0:AB + WP2 + 1], 0.0)
            else:
                nc.gpsimd.memset(act1[:, AB + (HPAD - 1) * WP2:BUF2], 0.0)
            nc.gpsimd.memset(ar[:, rr0:rr1, 0], 0.0)
            nc.gpsimd.memset(ar[:, rr0:rr1, 57:64], 0.0)
            # shifted duplicate for the (dy=0,dx=-1)+(dx=+1) DR pair, made by
            # SBUF->SBUF DMA on the gpsimd SW-DGE queue (borders included —
            # source already zeroed; keeps the HWDGE queues for HBM traffic)
            if h == 0:
                nc.gpsimd.dma_start(act1[:, ACT_D:ACT_D + HSPLIT],
                                    act1[:, 0:HSPLIT])
            else:
                nc.gpsimd.dma_start(act1[:, ACT_D + HSPLIT:ACT_D + BUF2],
                                    act1[:, HSPLIT:BUF2])

        def emit_B(n, h, tiles, cs=tuple(range(CPH))):
            """conv2 + bn2 + residual + qact for half h of image n (chunks
            cs of that half)."""
            x, xs = tiles[n]
            v1, act1 = tiles[(n, "a")]
            if (n, "out") not in tiles:
                v2 = sb.tile([C, BUF2], f32, tag="v2", name=f"v2_{n}")
                ost = sb.tile([C, H * W], f32, tag="ost", name=f"ost_{n}")
                tiles[(n, "out")] = (v2, ost)
            v2, ost = tiles[(n, "out")]
            rr0 = 1 + ROWS_H * h
            vr = v2[:, XB:XB + IMG2].rearrange("p (h w) -> p h w", w=WP2)
            xr = x[:, XBASE:XBASE + IMG2].rearrange("p (h w) -> p h w", w=WP2)
            ostr = ost.rearrange("p (h w) -> p h w", w=W)
            od_flat = out_d.ap()[n].rearrange("p h w -> p (h w)")
            last = (n == BPC - 1)

            def emit_tail(r0, r1, qi):
                # residual + qact + /15 + out-DMA for padded rows [r0, r1)
                # (for the last image the residual and Relu already happened
                # in PSUM / the bn2 affine)
                vq = vr[:, r0:r1, 0:WP]
                if not last:
                    nc.vector.tensor_add(vq, vq, xr[:, r0:r1, 0:WP])
                    nc.scalar.activation(vq, vq, AF.Relu)
                nc.vector.tensor_scalar(vq, vq, 15.0, MAGIC,
                                        op0=OP.min, op1=OP.add)
                nc.vector.tensor_scalar(ostr[:, r0 - 1:r1 - 1, :],
                                        vr[:, r0:r1, 1:57], MAGIC, inv15,
                                        op0=OP.subtract, op1=OP.mult)
                nc.sync.dma_start(od_flat[:, (r0 - 1) * W:(r1 - 1) * W],
                                  ost[:, (r0 - 1) * W:(r1 - 1) * W])

            # the last image's conv2 halves have no conv1 successor: h0 can
            # take the freed conv1 banks so its matmuls don't wait on h(-1)'s
            # affine2 chain, and the tail runs at chunk granularity to
            # shorten the end-of-kernel elementwise drain
            ptag = "pa" if (last and h == 0) else "pb"
            pb = {c: ps.tile([C, FREE2], f32, tag=ptag, bufs=CPH,
                             name=f"pb_{n}_{h}_{c}") for c in cs}
            # 3 DR pairs (dy=-1,+1 per dx); one real weight load per pair
            # (the dedupe pass below elides the repeats)
            for p, dx in enumerate((-1, 0, 1)):
                for c in cs:
                    r0 = rr0 + RPC * c
                    off_a = AB + (r0 - 1) * WP2 + dx
                    mv = bass.AP(tensor=act1.tensor, offset=off_a,
                                 ap=[[ABUF, C], [2 * WP2, 2], [WP2, RPC],
                                     [1, WP]])
                    nc.tensor.matmul(pb[c][:], lhsT=wpair(p), rhs=mv,
                                     perf_mode=DR, start=(p == 0), stop=False)
            # 4th DR pair: (dy=0,dx=-1) original + (dy=0,dx=+1) shifted copy
            for c in cs:
                r0 = rr0 + RPC * c
                off_a = AB + r0 * WP2 - 1
                mv = bass.AP(tensor=act1.tensor, offset=off_a,
                             ap=[[ABUF, C], [ACT_D + 2, 2], [WP2, RPC],
                                 [1, WP]])
                nc.tensor.matmul(pb[c][:], lhsT=wpair(3), rhs=mv,
                                 perf_mode=DR, start=False, stop=False)
            # center tap, plain fp8
            for c in cs:
                r0 = rr0 + RPC * c
                off = AB + r0 * WP2
                mv = bass.AP(tensor=act1.tensor, offset=off,
                             ap=[[ABUF, C], [WP2, RPC], [1, WP]])
                nc.tensor.matmul(pb[c][:], lhsT=w2p_sb[:, 8 * C:9 * C],
                                 rhs=mv, start=False, stop=not last)
            if last:
                # inject the residual into PSUM as diag(15/s2) @ x so the
                # end-of-kernel tail skips the DVE add and the standalone
                # Relu (folded into the bn2 affine below)
                for c in cs:
                    r0 = rr0 + RPC * c
                    off = XBASE + r0 * WP2
                    mv = bass.AP(tensor=x.tensor, offset=off,
                                 ap=[[BUFX, C], [WP2, RPC], [1, WP]])
                    nc.tensor.matmul(pb[c][:], lhsT=wdg_sb[:], rhs=mv,
                                     start=False, stop=True)
            # bn2 affine + the elementwise tail (14-row quarters in steady
            # state, single 7-row chunks for the last image)
            for c in cs:
                r0 = rr0 + RPC * c
                dst = v2[:, XB + r0 * WP2:XB + (r0 + RPC) * WP2].rearrange(
                    "p (h w) -> p h w", w=WP2)[:, :, 0:WP]
                nc.scalar.activation(dst,
                                     pb[c][:].rearrange(
                                         "p (h w) -> p h w", w=WP),
                                     AF.Relu if last else AF.Identity,
                                     bias=bi2_sb, scale=sc2_sb)
                if last and c % 2 == 1:
                    emit_tail(r0 - RPC, r0 + RPC, 2 * h + c // 2)
                elif last:
                    pass
                elif c == 1:
                    emit_tail(rr0, rr0 + 14, 2 * h)
                elif c == 3:
                    emit_tail(rr0 + 14, rr0 + 28, 2 * h + 1)

        tiles = {}
        for n in range(BPC):
            if n == 0:
                emit_load(0, 0, tiles)
                emit_load(0, 1, tiles)
            emit_A(n, 0, tiles)
            if n > 0:
                emit_B(n - 1, 0, tiles)
            if n + 1 < BPC:
                # queue the next image's input while this one computes (and
                # ahead of this image's conv1-dependent DVE work for h1)
                emit_load(n + 1, 0, tiles)
                emit_load(n + 1, 1, tiles)
            emit_A(n, 1, tiles)
            if n > 0:
                emit_B(n - 1, 1, tiles)
        emit_B(BPC - 1, 0, tiles)
        # last half in two 2-chunk sub-phases so the first sub-phase's
        # elementwise tail drains during the second's matmuls
        emit_B(BPC - 1, 1, tiles, cs=(0, 1))
        emit_B(BPC - 1, 1, tiles, cs=(2, 3))

    import concourse.mybir as mybir2
    _dedupe_ldweights(nc, mybir2)
    _fix_waits(nc, mybir2)
    return nc


def _get_module():
    if "nc" not in _CACHE:
        _CACHE["nc"] = _build_module()
    return _CACHE["nc"]


# ---------------------------------------------------------------- host entry
def _make_in_maps(x, w1, w2, gamma1, beta1, mean1, var1,
                  gamma2, beta2, mean2, var2):
    x15 = (np.float32(15.0) * np.asarray(x, np.float32)).astype(np.float16)
    x15 = x15.reshape(N_CORES, BPC, C, H, W)

    w1i = _quant_weight_int(np.asarray(w1, np.float32))
    w2i = _quant_weight_int(np.asarray(w2, np.float32))
    w2t = _lhsT_taps(w2i)  # [C, 9*C], tap-major (t9 = (dy+1)*3 + dx+1)
    tap = lambda t9: w2t[:, t9 * C:(t9 + 1) * C]
    blocks = []
    for dxi in range(3):           # DR pairs: (dy=-1,dx) then (dy=+1,dx)
        blocks += [tap(dxi), tap(6 + dxi)]
    blocks += [tap(3), tap(5)]     # DR pair: (dy=0,dx=-1) + (dy=0,dx=+1)
    blocks.append(tap(4))          # single: (dy=0,dx=0)
    w2p = np.concatenate(blocks, axis=1).astype(ml_dtypes.float8_e4m3)

    s1, b1 = _bn_affine(np.asarray(gamma1, np.float32), np.asarray(beta1, np.float32),
                        np.asarray(mean1, np.float32), np.asarray(var1, np.float32))
    s2, b2 = _bn_affine(np.asarray(gamma2, np.float32), np.asarray(beta2, np.float32),
                        np.asarray(mean2, np.float32), np.asarray(var2, np.float32))
    # conv PSUM holds 225*conv (15x-or-15a input, 15w weights) -> want 15*bn
    bnv = np.stack([s1 / 15.0, 15.0 * b1, s2 / 15.0, 15.0 * b2],
                   axis=1).astype(np.float32)  # [C, 4]

    w1r = _lhsT_taps(w1i).astype(np.float16)
    wdg = np.diag(np.float64(15.0) / s2).astype(np.float16)
    shared = {"w2p": w2p, "w1r": w1r, "bnv": bnv, "wdg": wdg}
    return [{"x15": np.ascontiguousarray(x15[i]), **shared}
            for i in range(N_CORES)]


def kernel(**inputs):
    from concourse.bass_utils import run_bass_kernel_spmd

    nc = _get_module()
    in_maps = _make_in_maps(**inputs)
    res = run_bass_kernel_spmd(nc, in_maps, core_ids=list(range(N_CORES)))
    _CACHE["last_res"] = res
    out = np.concatenate([np.asarray(r["out"], np.float32)
                          for r in res.results], axis=0)
    return out.reshape(B, C, H, W)


# revision 44
# speedup vs baseline: 1.0011x; 1.0011x over previous
"""Trainium2 Bass kernel for a DoReFa-quantized ResNet BasicBlock (inference).

Reference computation (all fp32):
    out = qact(bn2(conv3x3(qact(bn1(conv3x3(x, qw(w1)))), qw(w2))) + x)
with qw = 4-bit DoReFa weight quant, qact = 4-bit activation quant,
x: (64, 128, 56, 56), convs 128->128 stride 1 pad 1.

Sharding: data-parallel over the batch dim, 8 images per NeuronCore on 8 cores.

Per-core kernel design:
  * NCHW with C=128 on SBUF partitions, flattened zero-padded image rows in
    the free dim; a 3x3 conv = shifted 128x128 matmuls accumulated in PSUM.
    Half-image granularity: 28 output rows per half, 4 PSUM chunks of 7 rows
    (conv1 4 banks + conv2 4 banks = all 8).
  * conv1 runs in fp16 (x15 = 15*x shipped as fp16 from the host — the
    11-bit significand doubles the rounding of the f32r original but stays
    ~2x under the error gate, halves input HBM traffic, and streams at
    ~171 ns/392-col matmul vs 184 for f32r), tap-outer over 4 chunk
    matmuls per weight load.
    Quantized weights are exact small integers (15*w_q odd in [-15,15]) and
    activations 15*a in {0..15} (exact in fp8e4m3) -> conv2 is bit-exact
    integer fp8: 3 DoubleRow matmuls for the (dy=-1,+1) tap pairs, a 4th DR
    pair (dy=0,dx=-1)+(dx=+1) against a 16B-aligned shifted duplicate of
    act1 (made by SBUF->SBUF DMA on the gpsimd SW-DGE queue), 1 plain fp8
    matmul for the center tap. Measured on this part DR runs the array at
    1.2 GHz (vs 2.4 for normal mode), so DR's win is taps-per-pass, not
    cycles; pass-outer over 4 chunks amortizes the DR weight load.
  * A post-Tile pass rewrites the legalized module: redundant InstLdweights
    (same stationary operand as the previous load) become same-name NoOps,
    and repeated self-loading f32r matmuls get ldweights=False (validated
    bit-exact on HW); a second pass hoists walrus-illegal waits (>1 per
    instruction, any on a non-self-loading matmul) onto those NoOps.
  * BN folds into ScalarE activations out of PSUM (bn1 with Relu = the lower
    clip of the DoReFa staircase); staircase = tensor_scalar (min,+2^23) and
    (-2^23[,*1/15]) on VectorE, bit-matching jnp.round. qact1's second pass
    re-layouts W58 rows into the W64 act1 pitch for the fp8 DR pair stride.
  * Emission: load(n+1) queued while image n computes; PE order
    A(n,0) B(n-1,0) A(n,1) B(n-1,1) keeps the PE busy across the act1
    handoff. Steady-state input+output DMAs ride the dedicated sync HWDGE
    queue (a DMA trigger on the scalar engine blocks it head-of-line until
    the queue drains, starving the BN affines); only the first two images
    split across sync+scalar for the fill, with conv1 weights wrapped
    around image 0's chunks. The ScalarE activation table is pre-warmed
    with dummy ops so its lazy 1.3us load lands in the DMA fill.
  * Last image: conv2-h0 takes the freed conv1 PSUM banks, h1 runs as two
    2-chunk sub-phases, the residual is injected in PSUM via a diag(15/s2)
    f32r matmul with Relu folded into the bn2 affine, and the end-of-kernel
    drain is 2 DVE ops per 14-row piece.

Measured (8 cores, NTFF profile): ~181-183 us HW exec (baseline 194.1),
rel L2 err ~1.06e-2 (~1.1% of outputs off by one 1/15 quantization step;
gate 2e-2). PE ~162 us active: conv1 576 matmuls x ~171 ns, conv2 320 x
~177 ns (fp8 DoubleRow runs the array at 1.2 GHz, so DR's win is
taps-per-pass), plus ~7 us fixed preamble, ~4.5 us first-image DMA fill,
~8 us elementwise tail + teardown.
"""

import os
import sys

import numpy as np

for _p in ("/opt/trn_rl_repo", "/opt/pypackages"):
    if _p not in sys.path and os.path.isdir(_p):
        sys.path.insert(0, _p)

import ml_dtypes  # noqa: E402

# ---------------------------------------------------------------- constants
B, C, H, W = 64, 128, 56, 56
N_CORES = 8
BPC = B // N_CORES          # images per core
WP = W + 2                  # conv1/v1 padded row length (58)
WP2 = 64                    # conv2/act1/x padded row length (58 used + 6 dead)
HPAD = H + 2                # padded rows (58)
IMG = WP * HPAD             # v1-layout padded image elems (3364)
IMG2 = WP2 * HPAD           # act1/x-layout padded image elems (3712)
BUF = IMG + 4               # v1 buffer
BUF2 = IMG2 + 4             # act1/v2 buffer
XBASE = 1                   # x buffer base offset
BUFX = XBASE + IMG + 3      # x buffer (58-pitch)
ACT_D = 3726                # shifted act1 copy offset; pair step D+2 %16==0
ABUF = ACT_D + BUF2         # act1 tile width (original + shifted copy)
XB = 1                      # x / v buffers: image base offset
AB = 2                      # act1 buffer: base offset
RPC = 7                     # padded rows per PSUM chunk
CPH = 4                     # chunks per half (28 rows)
ROWS_H = RPC * CPH          # 28
FREE1 = RPC * W             # 392 free elems per conv1 matmul
FREE2 = RPC * WP            # 406 free elems per conv2 matmul
MAGIC = float(2**23)        # fp32 round-to-nearest-even magic constant
EPS = 1e-5

# input DMA row chunks (unpadded row ranges) per half
DMA_CHUNKS = (((0, 15), (15, 29)), ((29, 43), (43, 56)))

_CACHE = {}


def _dedupe_ldweights(nc, mybir):
    """Drop PE weight reloads whose stationary operand is identical to the
    immediately preceding load.

    Tile legalization pairs every fp8 InstMatmult with its own InstLdweights;
    with the tap-outer loop structure here, runs of 4 consecutive matmuls
    share the same weights, so 3 of every 4 loads are redundant. Replace each
    redundant InstLdweights with a same-name NoOp carrying its sync_info
    (keeps dependency edges and semaphore behavior intact). Self-loading
    matmuls (f32/f32r, which legalize does not split) clobber the array, so
    they reset the tracked key."""
    n = 0
    for fn in nc.m.functions:
        for blk in fn.blocks:
            last_key = None
            out = []
            for ins in blk.instructions:
                if ins.engine != mybir.EngineType.PE:
                    out.append(ins)
                    continue
                op = ins.opcode
                if op == "Ldweights":
                    key = (str(ins.ins[0]), str(ins.perf_mode),
                           str(ins.is_transpose))
                    if key == last_key:
                        nop = mybir.InstNoOp(name=ins.name, engine=ins.engine)
                        nop.sync_info = ins.sync_info
                        out.append(nop)
                        n += 1
                        continue
                    last_key = key
                elif op == "Matmult":
                    if getattr(ins, "ldweights", None) is not False:
                        # self-loading matmul (f32/f32r): if it repeats the
                        # previous stationary operand, skip its in-matmul
                        # weight load (validated on HW for f32r)
                        key = (str(ins.ins[1]), str(ins.perf_mode),
                               str(ins.is_transpose))
                        if key == last_key:
                            ins.ldweights = False
                            n += 1
                        else:
                            last_key = key
                elif op not in ("NoOp", "EventSemaphore"):
                    last_key = None
                out.append(ins)
            blk.instructions = out
    return n


# ---------------------------------------------------------------- host math
def _quant_weight_int(w):
    """Return 15*quantize_weight(w, 4) which is an exact odd integer in
    [-15, 15], as float32. Mirrors reference elementwise fp32 ops; tanh is
    computed in f64 and rounded (closest to any correctly-rounded f32 tanh)."""
    wt = np.tanh(w.astype(np.float64)).astype(np.float32)
    m = np.float32(np.abs(wt).max())
    wtn = wt / (np.float32(2.0) * m) + np.float32(0.5)      # [0, 1]
    q = np.round(wtn * np.float32(15.0)).astype(np.float32)  # {0..15}, half-even
    return np.float32(2.0) * q - np.float32(15.0)            # odd ints [-15,15]


def _bn_affine(gamma, beta, mean, var):
    """Per-channel (scale, bias) with bn(y) = scale*y + bias, in f64."""
    inv = 1.0 / np.sqrt(var.astype(np.float64) + EPS)
    s = gamma.astype(np.float64) * inv
    b = beta.astype(np.float64) - mean.astype(np.float64) * s
    return s, b


def _lhsT_taps(w_int):
    """[oc, ic, 3, 3] -> [ic, 9*oc] stationary-operand layout (tap-major)."""
    t = np.transpose(w_int, (2, 3, 1, 0)).reshape(9, C, C)   # [tap, ic, oc]
    return np.transpose(t, (1, 0, 2)).reshape(C, 9 * C)


# ---------------------------------------------------------------- bass build
def _fix_waits(nc, mybir):
    """Walrus encodes at most ONE sync wait per instruction, and none on a
    matmul marked non-self-loading by _dedupe_ldweights (its wait slot lives
    in the elided weight-load microcode).

    Hoist excess waits onto same-engine NoOps placed immediately before the
    instruction (the sequencer executes them in order, so semantics are
    unchanged)."""
    nid = 0
    for fn in nc.m.functions:
        for blk in fn.blocks:
            out = []
            changed = False
            for ins in blk.instructions:
                si = ins.sync_info
                noldw = (ins.opcode == "Matmult"
                         and getattr(ins, "ldweights", None) is False)
                if si is not None and (
                        len(si.on_wait) > 1 or (noldw and si.on_wait)):
                    waits = list(si.on_wait)
                    keep = [] if noldw else waits[-1:]
                    hoist = waits if noldw else waits[:-1]
                    for w in hoist:
                        nid += 1
                        nop = mybir.InstNoOp(name=f"I-wfix-{nid}",
                                             engine=ins.engine)
                        nop.sync_info = mybir.SyncInfo(on_wait=[w],
                                                       on_update=[])
                        out.append(nop)
                    ins.sync_info = mybir.SyncInfo(
                        on_wait=keep, on_update=list(si.on_update))
                    changed = True
                out.append(ins)
            if changed:
                blk.instructions = out


def _build_module():
    import concourse.bass as bass
    import concourse.mybir as mybir
    import concourse.tile as tile
    from contextlib import ExitStack

    f32 = mybir.dt.float32
    f16 = mybir.dt.float16
    f32r = mybir.dt.float32r
    f8 = mybir.dt.float8e4
    AF = mybir.ActivationFunctionType
    OP = mybir.AluOpType
    DR = mybir.MatmulPerfMode.DoubleRow

    nc = bass.Bass("TRN2", target_bir_lowering=False, debug=False,
                   num_devices=N_CORES)

    x_d = nc.dram_tensor("x15", [BPC, C, H, W], f16, kind="ExternalInput")
    # conv2 weights fp8: 3 DoubleRow pair blocks [2,128] (dy=-1/+1 per dx),
    # then the (dy=0,dx=-1)+(dx=+1) pair, then the center tap
    w2p_d = nc.dram_tensor("w2p", [C, 9 * C], f8, kind="ExternalInput")
    w1r_d = nc.dram_tensor("w1r", [C, 9 * C], f16, kind="ExternalInput")
    # columns: [sc1, bi1, sc2, bi2]
    bn_d = nc.dram_tensor("bnv", [C, 4], f32, kind="ExternalInput")
    # diag(15/s2) f32r: last-image residual injected via matmul into PSUM
    wdg_d = nc.dram_tensor("wdg", [C, C], f16, kind="ExternalInput")
    out_d = nc.dram_tensor("out", [BPC, C, H, W], f32, kind="ExternalOutput")

    inv15 = float(np.float32(1.0) / np.float32(15.0))
    HSPLIT = AB + (1 + ROWS_H) * WP2   # act1 byte split between halves

    with tile.TileContext(nc) as tc, ExitStack() as ctx:
        const = ctx.enter_context(tc.tile_pool(name="const", bufs=1))
        sb = ctx.enter_context(tc.tile_pool(name="sb", bufs=2))
        xp = ctx.enter_context(tc.tile_pool(name="xp", bufs=3))
        ps = ctx.enter_context(tc.tile_pool(name="ps", bufs=4, space="PSUM"))

        # conv1 weights: taps 0-2 up front on the scalar HWDGE queue (the
        # first matmul needs them); taps 3-8 deferred behind image 0's input
        w1r_sb = const.tile([C, 9 * C], f16)
        nc.scalar.dma_start(w1r_sb[:, 0:3 * C], w1r_d.ap()[:, 0:3 * C])
        wdg_sb = const.tile([C, C], f16)
        warm = const.tile([C, 128], f16)
        warmf = const.tile([C, 1], f32)
        nc.gpsimd.memset(warm[:], 0.0)
        nc.gpsimd.memset(warmf[:], 0.0)
        nc.scalar.activation(warmf[:], warmf[:], AF.Relu)
        nc.scalar.activation(warmf[:], warmf[:], AF.Identity)
        bn_sb = const.tile([C, 4], f32)
        w2p_sb = const.tile([C, 9 * C], f8)
        sc1_sb = bn_sb[:, 0:1]
        bi1_sb = bn_sb[:, 1:2]
        sc2_sb = bn_sb[:, 2:3]
        bi2_sb = bn_sb[:, 3:4]

        def wpair(p):
            return w2p_sb[:, p * 2 * C:(p + 1) * 2 * C].rearrange(
                "p (two m) -> p two m", two=2)

        def emit_load(n, h, tiles):
            """Input DMA + zero-padding for half h of image n."""
            if h == 0:
                x = xp.tile([C, BUFX], f16, tag="x", name=f"x_{n}")
                xs = sb.tile([C, H * W], f16, tag="xs", name=f"xs_{n}")
                tiles[n] = (x, xs)
            x, xs = tiles[n]
            xwr = x[:, XBASE:XBASE + IMG].rearrange("p (h w) -> p h w", w=WP)
            xr0 = x[:, XBASE:XBASE + IMG].rearrange("p (h w) -> p h w", w=WP)

            # zero pad borders for this half's rows (+ head/tail slack)
            # h0's dy=+1 taps read x row 29's pad cols, so zero rows 1..29
            # here and 30..56 in h1 (row 57 is covered by the tail memset)
            pr0, pr1 = (1, 1 + ROWS_H + 1) if h == 0 else (1 + ROWS_H + 1, 57)
            if h == 0:
                nc.gpsimd.memset(x[:, 0:XBASE + WP + 1], 0.0)    # slack+row0
            else:
                nc.gpsimd.memset(x[:, XBASE + (HPAD - 1) * WP:BUFX], 0.0)
            nc.gpsimd.memset(xr0[:, pr0:pr1, 0], 0.0)            # left pad col
            nc.gpsimd.memset(xr0[:, pr0:pr1, 57], 0.0)           # right pad col

            # contiguous DMAs into staging (split across the two HWDGE
            # queues), then DVE pad-insert
            xsr = xs.rearrange("p (h w) -> p h w", w=W)
            xd_flat = x_d.ap()[n].rearrange("p h w -> p (h w)")
            for ci, (a, b) in enumerate(DMA_CHUNKS[h]):
                eng = nc.scalar if (ci == 1 and n <= 1) else nc.sync
                eng.dma_start(xs[:, a * W:b * W], xd_flat[:, a * W:b * W])
                nc.vector.tensor_copy(xwr[:, a + 1:b + 1, 1:57],
                                      xsr[:, a:b, :])

            if n == 0 and h == 0:
                # PE p-state pre-warm: dummy matmuls during the DMA fill so
                # the array clock is ramped when the real stream starts
                pw = ps.tile([C, 128], f32, tag="pb", bufs=CPH, name="pwarm")
                for _ in range(40):
                    nc.tensor.matmul(pw[:], lhsT=warm[:, 0:128],
                                     rhs=warm[:, 0:128], start=True, stop=True)
                for q in (1, 2):
                    nc.scalar.dma_start(w1r_sb[:, q * 3 * C:(q + 1) * 3 * C],
                                        w1r_d.ap()[:, q * 3 * C:(q + 1) * 3 * C])
                # deferred const loads on the gpsimd SW-DGE queue: off both
                # HWDGE queues' critical paths (bn first read ~6us in, w2p
                # at the first conv2)
                nc.gpsimd.dma_start(bn_sb[:], bn_d.ap())
                nc.gpsimd.dma_start(w2p_sb[:], w2p_d.ap())
                nc.gpsimd.dma_start(wdg_sb[:], wdg_d.ap())

        def emit_A(n, h, tiles):
            """conv1 + bn1 + qact for half h of image n."""
            x, xs = tiles[n]
            if h == 0:
                v1 = sb.tile([C, BUF], f32, tag="v1", name=f"v1_{n}")
                act1 = sb.tile([C, ABUF], f8, tag="act1", name=f"act1_{n}")
                tiles[(n, "a")] = (v1, act1)
            v1, act1 = tiles[(n, "a")]
            rr0, rr1 = 1 + ROWS_H * h, 1 + ROWS_H * (h + 1)

            # conv1, tap-outer: one weight load per tap, 4 chunk matmuls
            pa = [ps.tile([C, FREE1], f32, tag="pa", bufs=CPH,
                          name=f"pa_{n}_{h}_{c}") for c in range(CPH)]
            for t9 in range(9):
                dy, dx = t9 // 3 - 1, t9 % 3 - 1
                for c in range(CPH):
                    r0 = rr0 + RPC * c
                    off = XBASE + (r0 + dy) * WP + 1 + dx
                    mv = bass.AP(tensor=x.tensor, offset=off,
                                 ap=[[BUFX, C], [WP, RPC], [1, W]])
                    nc.tensor.matmul(
                        pa[c][:], lhsT=w1r_sb[:, t9 * C:(t9 + 1) * C],
                        rhs=mv, start=(t9 == 0), stop=(t9 == 8))
            # bn1 affine out of PSUM; Relu = lower clip of the staircase
            for c in range(CPH):
                r0 = rr0 + RPC * c
                dst = v1[:, XB + r0 * WP:XB + (r0 + RPC) * WP].rearrange(
                    "p (h w) -> p h w", w=WP)[:, :, 1:57]
                nc.scalar.activation(dst,
                                     pa[c][:].rearrange(
                                         "p (h w) -> p h w", w=W),
                                     AF.Relu, bias=bi1_sb, scale=sc1_sb)

            # qact: min(.,15) then +M (one DVE pass), then -M into fp8 act1
            # (second pass also re-layouts W58 -> W64 rows)
            lo, hi = XB + rr0 * WP, XB + rr1 * WP
            nc.vector.tensor_scalar(v1[:, lo:hi], v1[:, lo:hi],
                                    15.0, MAGIC, op0=OP.min, op1=OP.add)
            v1r = v1[:, XB:XB + IMG].rearrange("p (h w) -> p h w", w=WP)
            ar = act1[:, AB:AB + IMG2].rearrange("p (h w) -> p h w", w=WP2)
            nc.vector.tensor_scalar(ar[:, rr0:rr1, 0:WP], v1r[:, rr0:rr1, 0:WP],
                                    MAGIC, None, op0=OP.subtract)
            if h == 0:
                nc.gpsimd.memset(act1[:, 0:AB + WP2 + 1], 0.0)
            else:
                nc.gpsimd.memset(act1[:, AB + (HPAD - 1) * WP2:BUF2], 0.0)
            nc.gpsimd.memset(ar[:, rr0:rr1, 0], 0.0)
            nc.gpsimd.memset(ar[:, rr0:rr1, 57:64], 0.0)
            # shifted duplicate for the (dy=0,dx=-1)+(dx=+1) DR pair, made by
            # SBUF->SBUF DMA on the gpsimd SW-DGE queue (borders included —
            # source already zeroed; keeps the HWDGE queues for HBM traffic)
            if h == 0:
                nc.gpsimd.dma_start(act1[:, ACT_D:ACT_D + HSPLIT],
                                    act1[:, 0:HSPLIT])
            else:
                nc.gpsimd.dma_start(act1[:, ACT_D + HSPLIT:ACT_D + BUF2],
                                    act1[:, HSPLIT:BUF2])

        def emit_B(n, h, tiles, cs=tuple(range(CPH))):
            """conv2 + bn2 + residual + qact for half h of image n (chunks
            cs of that half)."""
            x, xs = tiles[n]
            v1, act1 = tiles[(n, "a")]
            if (n, "out") not in tiles:
                v2 = sb.tile([C, BUF2], f32, tag="v2", name=f"v2_{n}")
                ost = sb.tile([C, H * W], f32, tag="ost", name=f"ost_{n}")
                tiles[(n, "out")] = (v2, ost)
            v2, ost = tiles[(n, "out")]
            rr0 = 1 + ROWS_H * h
            vr = v2[:, XB:XB + IMG2].rearrange("p (h w) -> p h w", w=WP2)
            xr = x[:, XBASE:XBASE + IMG].rearrange("p (h w) -> p h w", w=WP)
            ostr = ost.rearrange("p (h w) -> p h w", w=W)
            od_flat = out_d.ap()[n].rearrange("p h w -> p (h w)")
            last = (n == BPC - 1)

            def emit_tail(r0, r1, qi):
                # residual + qact + /15 + out-DMA for padded rows [r0, r1)
                # (for the last image the residual and Relu already happened
                # in PSUM / the bn2 affine)
                vq = vr[:, r0:r1, 0:WP]
                if not last:
                    nc.vector.tensor_add(vq, vq, xr[:, r0:r1, 0:WP])
                    nc.scalar.activation(vq, vq, AF.Relu)
                nc.vector.tensor_scalar(vq, vq, 15.0, MAGIC,
                                        op0=OP.min, op1=OP.add)
                nc.vector.tensor_scalar(ostr[:, r0 - 1:r1 - 1, :],
                                        vr[:, r0:r1, 1:57], MAGIC, inv15,
                                        op0=OP.subtract, op1=OP.mult)
                nc.sync.dma_start(od_flat[:, (r0 - 1) * W:(r1 - 1) * W],
                                  ost[:, (r0 - 1) * W:(r1 - 1) * W])

            # the last image's conv2 halves have no conv1 successor: h0 can
            # take the freed conv1 banks so its matmuls don't wait on h(-1)'s
            # affine2 chain, and the tail runs at chunk granularity to
            # shorten the end-of-kernel elementwise drain
            ptag = "pa" if (last and h == 0) else "pb"
            pb = {c: ps.tile([C, FREE2], f32, tag=ptag, bufs=CPH,
                             name=f"pb_{n}_{h}_{c}") for c in cs}
            # 3 DR pairs (dy=-1,+1 per dx); one real weight load per pair
            # (the dedupe pass below elides the repeats)
            for p, dx in enumerate((-1, 0, 1)):
                for c in cs:
                    r0 = rr0 + RPC * c
                    off_a = AB + (r0 - 1) * WP2 + dx
                    mv = bass.AP(tensor=act1.tensor, offset=off_a,
                                 ap=[[ABUF, C], [2 * WP2, 2], [WP2, RPC],
                                     [1, WP]])
                    nc.tensor.matmul(pb[c][:], lhsT=wpair(p), rhs=mv,
                                     perf_mode=DR, start=(p == 0), stop=False)
            # 4th DR pair: (dy=0,dx=-1) original + (dy=0,dx=+1) shifted copy
            for c in cs:
                r0 = rr0 + RPC * c
                off_a = AB + r0 * WP2 - 1
                mv = bass.AP(tensor=act1.tensor, offset=off_a,
                             ap=[[ABUF, C], [ACT_D + 2, 2], [WP2, RPC],
                                 [1, WP]])
                nc.tensor.matmul(pb[c][:], lhsT=wpair(3), rhs=mv,
                                 perf_mode=DR, start=False, stop=False)
            # center tap, plain fp8
            for c in cs:
                r0 = rr0 + RPC * c
                off = AB + r0 * WP2
                mv = bass.AP(tensor=act1.tensor, offset=off,
                             ap=[[ABUF, C], [WP2, RPC], [1, WP]])
                nc.tensor.matmul(pb[c][:], lhsT=w2p_sb[:, 8 * C:9 * C],
                                 rhs=mv, start=False, stop=not last)
            if last:
                # inject the residual into PSUM as diag(15/s2) @ x so the
                # end-of-kernel tail skips the DVE add and the standalone
                # Relu (folded into the bn2 affine below)
                for c in cs:
                    r0 = rr0 + RPC * c
                    off = XBASE + r0 * WP
                    mv = bass.AP(tensor=x.tensor, offset=off,
                                 ap=[[BUFX, C], [WP, RPC], [1, WP]])
                    nc.tensor.matmul(pb[c][:], lhsT=wdg_sb[:], rhs=mv,
                                     start=False, stop=True)
            # bn2 affine + the elementwise tail (14-row quarters in steady
            # state, single 7-row chunks for the last image)
            for c in cs:
                r0 = rr0 + RPC * c
                dst = v2[:, XB + r0 * WP2:XB + (r0 + RPC) * WP2].rearrange(
                    "p (h w) -> p h w", w=WP2)[:, :, 0:WP]
                nc.scalar.activation(dst,
                                     pb[c][:].rearrange(
                                         "p (h w) -> p h w", w=WP),
                                     AF.Relu if last else AF.Identity,
                                     bias=bi2_sb, scale=sc2_sb)
                if last and c % 2 == 1:
                    emit_tail(r0 - RPC, r0 + RPC, 2 * h + c // 2)
                elif last:
                    pass
                elif c == 1:
                    emit_tail(rr0, rr0 + 14, 2 * h)
                elif c == 3:
                    emit_tail(rr0 + 14, rr0 + 28, 2 * h + 1)

        tiles = {}
        for n in range(BPC):
            if n == 0:
                emit_load(0, 0, tiles)
                emit_load(0, 1, tiles)
            emit_A(n, 0, tiles)
            if n > 0:
                emit_B(n - 1, 0, tiles)
            if n + 1 < BPC:
                # queue the next image's input while this one computes (and
                # ahead of this image's conv1-dependent DVE work for h1)
                emit_load(n + 1, 0, tiles)
                emit_load(n + 1, 1, tiles)
            emit_A(n, 1, tiles)
            if n > 0:
                emit_B(n - 1, 1, tiles)
        emit_B(BPC - 1, 0, tiles)
        # last half in two 2-chunk sub-phases so the first sub-phase's
        # elementwise tail drains during the second's matmuls
        emit_B(BPC - 1, 1, tiles, cs=(0, 1))
        emit_B(BPC - 1, 1, tiles, cs=(2, 3))

    import concourse.mybir as mybir2
    _dedupe_ldweights(nc, mybir2)
    _fix_waits(nc, mybir2)
    return nc


def _get_module():
    if "nc" not in _CACHE:
        _CACHE["nc"] = _build_module()
    return _CACHE["nc"]


# ---------------------------------------------------------------- host entry
def _make_in_maps(x, w1, w2, gamma1, beta1, mean1, var1,
                  gamma2, beta2, mean2, var2):
    x15 = (np.float32(15.0) * np.asarray(x, np.float32)).astype(np.float16)
    x15 = x15.reshape(N_CORES, BPC, C, H, W)

    w1i = _quant_weight_int(np.asarray(w1, np.float32))
    w2i = _quant_weight_int(np.asarray(w2, np.float32))
    w2t = _lhsT_taps(w2i)  # [C, 9*C], tap-major (t9 = (dy+1)*3 + dx+1)
    tap = lambda t9: w2t[:, t9 * C:(t9 + 1) * C]
    blocks = []
    for dxi in range(3):           # DR pairs: (dy=-1,dx) then (dy=+1,dx)
        blocks += [tap(dxi), tap(6 + dxi)]
    blocks += [tap(3), tap(5)]     # DR pair: (dy=0,dx=-1) + (dy=0,dx=+1)
    blocks.append(tap(4))          # single: (dy=0,dx=0)
    w2p = np.concatenate(blocks, axis=1).astype(ml_dtypes.float8_e4m3)

    s1, b1 = _bn_affine(np.asarray(gamma1, np.float32), np.asarray(beta1, np.float32),
                        np.asarray(mean1, np.float32), np.asarray(var1, np.float32))
    s2, b2 = _bn_affine(np.asarray(gamma2, np.float32), np.asarray(beta2, np.float32),
                        np.asarray(mean2, np.float32), np.asarray(var2, np.float32))
    # conv PSUM holds 225*conv (15x-or-15a input, 15w weights) -> want 15*bn
    bnv = np.stack([s1 / 15.0, 15.0 * b1, s2 / 15.0, 15.0 * b2],
                   axis=1).astype(np.float32)  # [C, 4]

    w1r = _lhsT_taps(w1i).astype(np.float16)
    wdg = np.diag(np.float64(15.0) / s2).astype(np.float16)
    shared = {"w2p": w2p, "w1r": w1r, "bnv": bnv, "wdg": wdg}
    return [{"x15": np.ascontiguousarray(x15[i]), **shared}
            for i in range(N_CORES)]


def kernel(**inputs):
    from concourse.bass_utils import run_bass_kernel_spmd

    nc = _get_module()
    in_maps = _make_in_maps(**inputs)
    res = run_bass_kernel_spmd(nc, in_maps, core_ids=list(range(N_CORES)))
    _CACHE["last_res"] = res
    out = np.concatenate([np.asarray(r["out"], np.float32)
                          for r in res.results], axis=0)
    return out.reshape(B, C, H, W)


# revision 45
# speedup vs baseline: 1.0116x; 1.0105x over previous
"""Trainium2 Bass kernel for a DoReFa-quantized ResNet BasicBlock (inference).

Reference computation (all fp32):
    out = qact(bn2(conv3x3(qact(bn1(conv3x3(x, qw(w1)))), qw(w2))) + x)
with qw = 4-bit DoReFa weight quant, qact = 4-bit activation quant,
x: (64, 128, 56, 56), convs 128->128 stride 1 pad 1.

Sharding: data-parallel over the batch dim, 8 images per NeuronCore on 8 cores.

Per-core kernel design:
  * NCHW with C=128 on SBUF partitions, flattened zero-padded image rows in
    the free dim; a 3x3 conv = shifted 128x128 matmuls accumulated in PSUM.
    Half-image granularity: 28 output rows per half, 4 PSUM chunks of 7 rows
    (conv1 4 banks + conv2 4 banks = all 8).
  * conv1 runs in fp16 (x15 = 15*x shipped as fp16 from the host — the
    11-bit significand doubles the rounding of the f32r original but stays
    ~2x under the error gate, halves input HBM traffic, and streams at
    ~171 ns/392-col matmul vs 184 for f32r), tap-outer over 4 chunk
    matmuls per weight load.
    Quantized weights are exact small integers (15*w_q odd in [-15,15]) and
    activations 15*a in {0..15} (exact in fp8e4m3) -> conv2 is bit-exact
    integer fp8: 3 DoubleRow matmuls for the (dy=-1,+1) tap pairs, a 4th DR
    pair (dy=0,dx=-1)+(dx=+1) against a 16B-aligned shifted duplicate of
    act1 (made by SBUF->SBUF DMA on the gpsimd SW-DGE queue), 1 plain fp8
    matmul for the center tap. Measured on this part DR runs the array at
    1.2 GHz (vs 2.4 for normal mode), so DR's win is taps-per-pass, not
    cycles; pass-outer over 4 chunks amortizes the DR weight load.
  * A post-Tile pass rewrites the legalized module: redundant InstLdweights
    (same stationary operand as the previous load) become same-name NoOps,
    and repeated self-loading f32r matmuls get ldweights=False (validated
    bit-exact on HW); a second pass hoists walrus-illegal waits (>1 per
    instruction, any on a non-self-loading matmul) onto those NoOps.
  * BN folds into ScalarE activations out of PSUM (bn1 with Relu = the lower
    clip of the DoReFa staircase); staircase = tensor_scalar (min,+2^23) and
    (-2^23[,*1/15]) on VectorE, bit-matching jnp.round. qact1's second pass
    re-layouts W58 rows into the W64 act1 pitch for the fp8 DR pair stride.
  * Emission: load(n+1) queued while image n computes; PE order
    A(n,0) B(n-1,0) A(n,1) B(n-1,1) keeps the PE busy across the act1
    handoff. Steady-state input+output DMAs ride the dedicated sync HWDGE
    queue (a DMA trigger on the scalar engine blocks it head-of-line until
    the queue drains, starving the BN affines); only the first two images
    split across sync+scalar for the fill, with conv1 weights wrapped
    around image 0's chunks. The ScalarE activation table is pre-warmed
    with dummy ops so its lazy 1.3us load lands in the DMA fill.
  * Last image: conv2-h0 takes the freed conv1 PSUM banks, h1 runs as two
    2-chunk sub-phases, the residual is injected in PSUM via a diag(15/s2)
    f32r matmul with Relu folded into the bn2 affine, and the end-of-kernel
    drain is 2 DVE ops per 14-row piece.

Measured (8 cores, NTFF profile): ~181-183 us HW exec (baseline 194.1),
rel L2 err ~1.06e-2 (~1.1% of outputs off by one 1/15 quantization step;
gate 2e-2). PE ~162 us active: conv1 576 matmuls x ~171 ns, conv2 320 x
~177 ns (fp8 DoubleRow runs the array at 1.2 GHz, so DR's win is
taps-per-pass), plus ~7 us fixed preamble, ~4.5 us first-image DMA fill,
~8 us elementwise tail + teardown.
"""

import os
import sys

import numpy as np

for _p in ("/opt/trn_rl_repo", "/opt/pypackages"):
    if _p not in sys.path and os.path.isdir(_p):
        sys.path.insert(0, _p)

import ml_dtypes  # noqa: E402

# ---------------------------------------------------------------- constants
B, C, H, W = 64, 128, 56, 56
N_CORES = 8
BPC = B // N_CORES          # images per core
WP = W + 2                  # conv1/v1 padded row length (58)
WP2 = 64                    # conv2/act1/x padded row length (58 used + 6 dead)
HPAD = H + 2                # padded rows (58)
IMG = WP * HPAD             # v1-layout padded image elems (3364)
IMG2 = WP2 * HPAD           # act1/x-layout padded image elems (3712)
BUF = IMG + 4               # v1 buffer
BUF2 = IMG2 + 4             # act1/v2 buffer
XBASE = 1                   # x buffer base offset
BUFX = XBASE + IMG + 3      # x buffer (58-pitch)
ACT_D = 3726                # shifted act1 copy offset; pair step D+2 %16==0
ABUF = ACT_D + BUF2         # act1 tile width (original + shifted copy)
XB = 1                      # x / v buffers: image base offset
AB = 2                      # act1 buffer: base offset
RPC = 7                     # padded rows per PSUM chunk
CPH = 4                     # chunks per half (28 rows)
ROWS_H = RPC * CPH          # 28
FREE1 = RPC * W             # 392 free elems per conv1 matmul
FREE2 = RPC * WP            # 406 free elems per conv2 matmul
MAGIC = float(2**23)        # fp32 round-to-nearest-even magic constant
EPS = 1e-5

# input DMA row chunks (unpadded row ranges) per half
DMA_CHUNKS = (((0, 15), (15, 29)), ((29, 43), (43, 56)))

_CACHE = {}


def _dedupe_ldweights(nc, mybir):
    """Drop PE weight reloads whose stationary operand is identical to the
    immediately preceding load.

    Tile legalization pairs every fp8 InstMatmult with its own InstLdweights;
    with the tap-outer loop structure here, runs of 4 consecutive matmuls
    share the same weights, so 3 of every 4 loads are redundant. Replace each
    redundant InstLdweights with a same-name NoOp carrying its sync_info
    (keeps dependency edges and semaphore behavior intact). Self-loading
    matmuls (f32/f32r, which legalize does not split) clobber the array, so
    they reset the tracked key."""
    n = 0
    for fn in nc.m.functions:
        for blk in fn.blocks:
            last_key = None
            out = []
            for ins in blk.instructions:
                if ins.engine != mybir.EngineType.PE:
                    out.append(ins)
                    continue
                op = ins.opcode
                if op == "Ldweights":
                    key = (str(ins.ins[0]), str(ins.perf_mode),
                           str(ins.is_transpose))
                    if key == last_key:
                        nop = mybir.InstNoOp(name=ins.name, engine=ins.engine)
                        nop.sync_info = ins.sync_info
                        out.append(nop)
                        n += 1
                        continue
                    last_key = key
                elif op == "Matmult":
                    if getattr(ins, "ldweights", None) is not False:
                        # self-loading matmul (f32/f32r): if it repeats the
                        # previous stationary operand, skip its in-matmul
                        # weight load (validated on HW for f32r)
                        key = (str(ins.ins[1]), str(ins.perf_mode),
                               str(ins.is_transpose))
                        if key == last_key:
                            ins.ldweights = False
                            n += 1
                        else:
                            last_key = key
                elif op not in ("NoOp", "EventSemaphore"):
                    last_key = None
                out.append(ins)
            blk.instructions = out
    return n


# ---------------------------------------------------------------- host math
def _quant_weight_int(w):
    """Return 15*quantize_weight(w, 4) which is an exact odd integer in
    [-15, 15], as float32. Mirrors reference elementwise fp32 ops; tanh is
    computed in f64 and rounded (closest to any correctly-rounded f32 tanh)."""
    wt = np.tanh(w.astype(np.float64)).astype(np.float32)
    m = np.float32(np.abs(wt).max())
    wtn = wt / (np.float32(2.0) * m) + np.float32(0.5)      # [0, 1]
    q = np.round(wtn * np.float32(15.0)).astype(np.float32)  # {0..15}, half-even
    return np.float32(2.0) * q - np.float32(15.0)            # odd ints [-15,15]


def _bn_affine(gamma, beta, mean, var):
    """Per-channel (scale, bias) with bn(y) = scale*y + bias, in f64."""
    inv = 1.0 / np.sqrt(var.astype(np.float64) + EPS)
    s = gamma.astype(np.float64) * inv
    b = beta.astype(np.float64) - mean.astype(np.float64) * s
    return s, b


def _lhsT_taps(w_int):
    """[oc, ic, 3, 3] -> [ic, 9*oc] stationary-operand layout (tap-major)."""
    t = np.transpose(w_int, (2, 3, 1, 0)).reshape(9, C, C)   # [tap, ic, oc]
    return np.transpose(t, (1, 0, 2)).reshape(C, 9 * C)


# ---------------------------------------------------------------- bass build
def _fix_waits(nc, mybir):
    """Walrus encodes at most ONE sync wait per instruction, and none on a
    matmul marked non-self-loading by _dedupe_ldweights (its wait slot lives
    in the elided weight-load microcode).

    Hoist excess waits onto same-engine NoOps placed immediately before the
    instruction (the sequencer executes them in order, so semantics are
    unchanged)."""
    nid = 0
    for fn in nc.m.functions:
        for blk in fn.blocks:
            out = []
            changed = False
            for ins in blk.instructions:
                si = ins.sync_info
                noldw = (ins.opcode == "Matmult"
                         and getattr(ins, "ldweights", None) is False)
                if si is not None and (
                        len(si.on_wait) > 1 or (noldw and si.on_wait)):
                    waits = list(si.on_wait)
                    keep = [] if noldw else waits[-1:]
                    hoist = waits if noldw else waits[:-1]
                    for w in hoist:
                        nid += 1
                        nop = mybir.InstNoOp(name=f"I-wfix-{nid}",
                                             engine=ins.engine)
                        nop.sync_info = mybir.SyncInfo(on_wait=[w],
                                                       on_update=[])
                        out.append(nop)
                    ins.sync_info = mybir.SyncInfo(
                        on_wait=keep, on_update=list(si.on_update))
                    changed = True
                out.append(ins)
            if changed:
                blk.instructions = out


def _build_module():
    import concourse.bass as bass
    import concourse.mybir as mybir
    import concourse.tile as tile
    from contextlib import ExitStack

    f32 = mybir.dt.float32
    f16 = mybir.dt.float16
    f32r = mybir.dt.float32r
    f8 = mybir.dt.float8e4
    AF = mybir.ActivationFunctionType
    OP = mybir.AluOpType
    DR = mybir.MatmulPerfMode.DoubleRow

    nc = bass.Bass("TRN2", target_bir_lowering=False, debug=False,
                   num_devices=N_CORES)

    x_d = nc.dram_tensor("x15", [BPC, C, H, W], f16, kind="ExternalInput")
    # conv2 weights fp8: 3 DoubleRow pair blocks [2,128] (dy=-1/+1 per dx),
    # then the (dy=0,dx=-1)+(dx=+1) pair, then the center tap
    w2p_d = nc.dram_tensor("w2p", [C, 9 * C], f8, kind="ExternalInput")
    w1r_d = nc.dram_tensor("w1r", [C, 9 * C], f16, kind="ExternalInput")
    # columns: [sc1, bi1, sc2, bi2]
    bn_d = nc.dram_tensor("bnv", [C, 4], f32, kind="ExternalInput")
    # diag(15/s2) f32r: last-image residual injected via matmul into PSUM
    wdg_d = nc.dram_tensor("wdg", [C, C], f16, kind="ExternalInput")
    out_d = nc.dram_tensor("out", [BPC, C, H, W], f32, kind="ExternalOutput")

    inv15 = float(np.float32(1.0) / np.float32(15.0))
    HSPLIT = AB + (1 + ROWS_H) * WP2   # act1 byte split between halves

    with tile.TileContext(nc) as tc, ExitStack() as ctx:
        const = ctx.enter_context(tc.tile_pool(name="const", bufs=1))
        sb = ctx.enter_context(tc.tile_pool(name="sb", bufs=2))
        xp = ctx.enter_context(tc.tile_pool(name="xp", bufs=3))
        ps = ctx.enter_context(tc.tile_pool(name="ps", bufs=4, space="PSUM"))

        # conv1 weights: taps 0-2 up front on the scalar HWDGE queue (the
        # first matmul needs them); taps 3-8 deferred behind image 0's input
        w1r_sb = const.tile([C, 9 * C], f16)
        nc.scalar.dma_start(w1r_sb[:, 0:3 * C], w1r_d.ap()[:, 0:3 * C])
        wdg_sb = const.tile([C, C], f16)
        warm = const.tile([C, 128], f16)
        warmf = const.tile([C, 1], f32)
        nc.gpsimd.memset(warm[:], 0.0)
        nc.gpsimd.memset(warmf[:], 0.0)
        nc.scalar.activation(warmf[:], warmf[:], AF.Relu)
        nc.scalar.activation(warmf[:], warmf[:], AF.Identity)
        bn_sb = const.tile([C, 4], f32)
        w2p_sb = const.tile([C, 9 * C], f8)
        sc1_sb = bn_sb[:, 0:1]
        bi1_sb = bn_sb[:, 1:2]
        sc2_sb = bn_sb[:, 2:3]
        bi2_sb = bn_sb[:, 3:4]

        def wpair(p):
            return w2p_sb[:, p * 2 * C:(p + 1) * 2 * C].rearrange(
                "p (two m) -> p two m", two=2)

        def emit_load(n, h, tiles):
            """Input DMA + zero-padding for half h of image n."""
            if h == 0:
                x = xp.tile([C, BUFX], f16, tag="x", name=f"x_{n}")
                xs = sb.tile([C, H * W], f16, tag="xs", name=f"xs_{n}")
                tiles[n] = (x, xs)
            x, xs = tiles[n]
            xwr = x[:, XBASE:XBASE + IMG].rearrange("p (h w) -> p h w", w=WP)
            xr0 = x[:, XBASE:XBASE + IMG].rearrange("p (h w) -> p h w", w=WP)

            # zero pad borders for this half's rows (+ head/tail slack)
            # h0's dy=+1 taps read x row 29's pad cols, so zero rows 1..29
            # here and 30..56 in h1 (row 57 is covered by the tail memset)
            pr0, pr1 = (1, 1 + ROWS_H + 1) if h == 0 else (1 + ROWS_H + 1, 57)
            if h == 0:
                nc.gpsimd.memset(x[:, 0:XBASE + WP + 1], 0.0)    # slack+row0
            else:
                nc.gpsimd.memset(x[:, XBASE + (HPAD - 1) * WP:BUFX], 0.0)
            nc.gpsimd.memset(xr0[:, pr0:pr1, 0], 0.0)            # left pad col
            nc.gpsimd.memset(xr0[:, pr0:pr1, 57], 0.0)           # right pad col

            # contiguous DMAs into staging (split across the two HWDGE
            # queues), then DVE pad-insert
            xsr = xs.rearrange("p (h w) -> p h w", w=W)
            xd_flat = x_d.ap()[n].rearrange("p h w -> p (h w)")
            for ci, (a, b) in enumerate(DMA_CHUNKS[h]):
                eng = nc.scalar if (ci == 1 and n <= 1) else nc.sync
                eng.dma_start(xs[:, a * W:b * W], xd_flat[:, a * W:b * W])
                nc.vector.tensor_copy(xwr[:, a + 1:b + 1, 1:57],
                                      xsr[:, a:b, :])

            if n == 0 and h == 0:
                # PE p-state pre-warm: dummy matmuls during the DMA fill so
                # the array clock is ramped when the real stream starts
                pw = ps.tile([C, 128], f32, tag="pb", bufs=CPH, name="pwarm")
                for _ in range(40):
                    nc.tensor.matmul(pw[:], lhsT=warm[:, 0:128],
                                     rhs=warm[:, 0:128], start=True, stop=True)
                for q in (1, 2):
                    nc.scalar.dma_start(w1r_sb[:, q * 3 * C:(q + 1) * 3 * C],
                                        w1r_d.ap()[:, q * 3 * C:(q + 1) * 3 * C])
                # deferred const loads on the gpsimd SW-DGE queue: off both
                # HWDGE queues' critical paths (bn first read ~6us in, w2p
                # at the first conv2)
                nc.gpsimd.dma_start(bn_sb[:], bn_d.ap())
                nc.gpsimd.dma_start(w2p_sb[:], w2p_d.ap())
                nc.gpsimd.dma_start(wdg_sb[:], wdg_d.ap())

        def emit_A(n, h, tiles):
            """conv1 + bn1 + qact for half h of image n."""
            x, xs = tiles[n]
            if h == 0:
                v1 = sb.tile([C, BUF], f32, tag="v1", name=f"v1_{n}")
                act1 = sb.tile([C, ABUF], f8, tag="act1", name=f"act1_{n}")
                tiles[(n, "a")] = (v1, act1)
            v1, act1 = tiles[(n, "a")]
            rr0, rr1 = 1 + ROWS_H * h, 1 + ROWS_H * (h + 1)

            # conv1, tap-outer: one weight load per tap, 4 chunk matmuls
            pa = [ps.tile([C, FREE1], f32, tag="pa", bufs=CPH,
                          name=f"pa_{n}_{h}_{c}") for c in range(CPH)]
            for t9 in range(9):
                dy, dx = t9 // 3 - 1, t9 % 3 - 1
                for c in range(CPH):
                    r0 = rr0 + RPC * c
                    off = XBASE + (r0 + dy) * WP + 1 + dx
                    mv = bass.AP(tensor=x.tensor, offset=off,
                                 ap=[[BUFX, C], [WP, RPC], [1, W]])
                    nc.tensor.matmul(
                        pa[c][:], lhsT=w1r_sb[:, t9 * C:(t9 + 1) * C],
                        rhs=mv, start=(t9 == 0), stop=(t9 == 8))
            # bn1 affine out of PSUM; Relu = lower clip of the staircase
            for c in range(CPH):
                r0 = rr0 + RPC * c
                dst = v1[:, XB + r0 * WP:XB + (r0 + RPC) * WP].rearrange(
                    "p (h w) -> p h w", w=WP)[:, :, 1:57]
                nc.scalar.activation(dst,
                                     pa[c][:].rearrange(
                                         "p (h w) -> p h w", w=W),
                                     AF.Relu, bias=bi1_sb, scale=sc1_sb)

            # qact: min(.,15) then +M (one DVE pass), then -M into fp8 act1
            # (second pass also re-layouts W58 -> W64 rows)
            lo, hi = XB + rr0 * WP, XB + rr1 * WP
            nc.vector.tensor_scalar(v1[:, lo:hi], v1[:, lo:hi],
                                    15.0, MAGIC, op0=OP.min, op1=OP.add)
            v1r = v1[:, XB:XB + IMG].rearrange("p (h w) -> p h w", w=WP)
            ar = act1[:, AB:AB + IMG2].rearrange("p (h w) -> p h w", w=WP2)
            nc.vector.tensor_scalar(ar[:, rr0:rr1, 0:WP], v1r[:, rr0:rr1, 0:WP],
                                    MAGIC, None, op0=OP.subtract)
            if h == 0:
                nc.gpsimd.memset(act1[:, 0:AB + WP2 + 1], 0.0)
            else:
                nc.gpsimd.memset(act1[:, AB + (HPAD - 1) * WP2:BUF2], 0.0)
            nc.gpsimd.memset(ar[:, rr0:rr1, 0], 0.0)
            nc.gpsimd.memset(ar[:, rr0:rr1, 57:64], 0.0)
            # shifted duplicate for the (dy=0,dx=-1)+(dx=+1) DR pair, made by
            # SBUF->SBUF DMA on the gpsimd SW-DGE queue (borders included —
            # source already zeroed; keeps the HWDGE queues for HBM traffic)
            if h == 0:
                nc.gpsimd.dma_start(act1[:, ACT_D:ACT_D + HSPLIT],
                                    act1[:, 0:HSPLIT])
            else:
                nc.gpsimd.dma_start(act1[:, ACT_D + HSPLIT:ACT_D + BUF2],
                                    act1[:, HSPLIT:BUF2])

        def emit_B(n, h, tiles, cs=tuple(range(CPH))):
            """conv2 + bn2 + residual + qact for half h of image n (chunks
            cs of that half)."""
            x, xs = tiles[n]
            v1, act1 = tiles[(n, "a")]
            if (n, "out") not in tiles:
                v2 = sb.tile([C, BUF2], f32, tag="v2", name=f"v2_{n}")
                ost = sb.tile([C, H * W], f32, tag="ost", name=f"ost_{n}")
                tiles[(n, "out")] = (v2, ost)
            v2, ost = tiles[(n, "out")]
            rr0 = 1 + ROWS_H * h
            vr = v2[:, XB:XB + IMG2].rearrange("p (h w) -> p h w", w=WP2)
            xr = x[:, XBASE:XBASE + IMG].rearrange("p (h w) -> p h w", w=WP)
            ostr = ost.rearrange("p (h w) -> p h w", w=W)
            od_flat = out_d.ap()[n].rearrange("p h w -> p (h w)")
            last = (n == BPC - 1)

            def emit_tail(r0, r1, qi):
                # residual + qact + /15 + out-DMA for padded rows [r0, r1)
                # (for the last image the residual and Relu already happened
                # in PSUM / the bn2 affine)
                vq = vr[:, r0:r1, 0:WP]
                if not last:
                    nc.vector.tensor_add(vq, vq, xr[:, r0:r1, 0:WP])
                    nc.scalar.activation(vq, vq, AF.Relu)
                nc.vector.tensor_scalar(vq, vq, 15.0, MAGIC,
                                        op0=OP.min, op1=OP.add)
                nc.vector.tensor_scalar(ostr[:, r0 - 1:r1 - 1, :],
                                        vr[:, r0:r1, 1:57], MAGIC, inv15,
                                        op0=OP.subtract, op1=OP.mult)
                nc.sync.dma_start(od_flat[:, (r0 - 1) * W:(r1 - 1) * W],
                                  ost[:, (r0 - 1) * W:(r1 - 1) * W])

            # the last image's conv2 halves have no conv1 successor: h0 can
            # take the freed conv1 banks so its matmuls don't wait on h(-1)'s
            # affine2 chain, and the tail runs at chunk granularity to
            # shorten the end-of-kernel elementwise drain
            ptag = "pa" if (last and h == 0) else "pb"
            pb = {c: ps.tile([C, FREE2], f32, tag=ptag, bufs=CPH,
                             name=f"pb_{n}_{h}_{c}") for c in cs}
            # 3 DR pairs (dy=-1,+1 per dx); one real weight load per pair
            # (the dedupe pass below elides the repeats)
            for p, dx in enumerate((-1, 0, 1)):
                for c in cs:
                    r0 = rr0 + RPC * c
                    off_a = AB + (r0 - 1) * WP2 + dx
                    mv = bass.AP(tensor=act1.tensor, offset=off_a,
                                 ap=[[ABUF, C], [2 * WP2, 2], [WP2, RPC],
                                     [1, WP]])
                    nc.tensor.matmul(pb[c][:], lhsT=wpair(p), rhs=mv,
                                     perf_mode=DR, start=(p == 0), stop=False)
            # 4th DR pair: (dy=0,dx=-1) original + (dy=0,dx=+1) shifted copy
            for c in cs:
                r0 = rr0 + RPC * c
                off_a = AB + r0 * WP2 - 1
                mv = bass.AP(tensor=act1.tensor, offset=off_a,
                             ap=[[ABUF, C], [ACT_D + 2, 2], [WP2, RPC],
                                 [1, WP]])
                nc.tensor.matmul(pb[c][:], lhsT=wpair(3), rhs=mv,
                                 perf_mode=DR, start=False, stop=False)
            # center tap, plain fp8
            for c in cs:
                r0 = rr0 + RPC * c
                off = AB + r0 * WP2
                mv = bass.AP(tensor=act1.tensor, offset=off,
                             ap=[[ABUF, C], [WP2, RPC], [1, WP]])
                nc.tensor.matmul(pb[c][:], lhsT=w2p_sb[:, 8 * C:9 * C],
                                 rhs=mv, start=False, stop=not last)
            if last:
                # inject the residual into PSUM as diag(15/s2) @ x so the
                # end-of-kernel tail skips the DVE add and the standalone
                # Relu (folded into the bn2 affine below)
                for c in cs:
                    r0 = rr0 + RPC * c
                    off = XBASE + r0 * WP
                    mv = bass.AP(tensor=x.tensor, offset=off,
                                 ap=[[BUFX, C], [WP, RPC], [1, WP]])
                    nc.tensor.matmul(pb[c][:], lhsT=wdg_sb[:], rhs=mv,
                                     start=False, stop=True)
            # bn2 affine + the elementwise tail (14-row quarters in steady
            # state, single 7-row chunks for the last image)
            for c in cs:
                r0 = rr0 + RPC * c
                dst = v2[:, XB + r0 * WP2:XB + (r0 + RPC) * WP2].rearrange(
                    "p (h w) -> p h w", w=WP2)[:, :, 0:WP]
                nc.scalar.activation(dst,
                                     pb[c][:].rearrange(
                                         "p (h w) -> p h w", w=WP),
                                     AF.Relu if last else AF.Identity,
                                     bias=bi2_sb, scale=sc2_sb)
                if last and c % 2 == 1:
                    emit_tail(r0 - RPC, r0 + RPC, 2 * h + c // 2)
                elif last:
                    pass
                elif c == 1:
                    emit_tail(rr0, rr0 + 14, 2 * h)
                elif c == 3:
                    emit_tail(rr0 + 14, rr0 + 28, 2 * h + 1)

        tiles = {}
        for n in range(BPC):
            if n == 0:
                emit_load(0, 0, tiles)
                emit_load(0, 1, tiles)
            emit_A(n, 0, tiles)
            if n > 0:
                emit_B(n - 1, 0, tiles)
            if n + 1 < BPC:
                # queue the next image's input while this one computes (and
                # ahead of this image's conv1-dependent DVE work for h1)
                emit_load(n + 1, 0, tiles)
                emit_load(n + 1, 1, tiles)
            emit_A(n, 1, tiles)
            if n > 0:
                emit_B(n - 1, 1, tiles)
        emit_B(BPC - 1, 0, tiles)
        # last half in two 2-chunk sub-phases so the first sub-phase's
        # elementwise tail drains during the second's matmuls
        emit_B(BPC - 1, 1, tiles, cs=(0, 1, 2))
        emit_B(BPC - 1, 1, tiles, cs=(3,))

    import concourse.mybir as mybir2
    _dedupe_ldweights(nc, mybir2)
    _fix_waits(nc, mybir2)
    return nc


def _get_module():
    if "nc" not in _CACHE:
        _CACHE["nc"] = _build_module()
    return _CACHE["nc"]


# ---------------------------------------------------------------- host entry
def _make_in_maps(x, w1, w2, gamma1, beta1, mean1, var1,
                  gamma2, beta2, mean2, var2):
    x15 = (np.float32(15.0) * np.asarray(x, np.float32)).astype(np.float16)
    x15 = x15.reshape(N_CORES, BPC, C, H, W)

    w1i = _quant_weight_int(np.asarray(w1, np.float32))
    w2i = _quant_weight_int(np.asarray(w2, np.float32))
    w2t = _lhsT_taps(w2i)  # [C, 9*C], tap-major (t9 = (dy+1)*3 + dx+1)
    tap = lambda t9: w2t[:, t9 * C:(t9 + 1) * C]
    blocks = []
    for dxi in range(3):           # DR pairs: (dy=-1,dx) then (dy=+1,dx)
        blocks += [tap(dxi), tap(6 + dxi)]
    blocks += [tap(3), tap(5)]     # DR pair: (dy=0,dx=-1) + (dy=0,dx=+1)
    blocks.append(tap(4))          # single: (dy=0,dx=0)
    w2p = np.concatenate(blocks, axis=1).astype(ml_dtypes.float8_e4m3)

    s1, b1 = _bn_affine(np.asarray(gamma1, np.float32), np.asarray(beta1, np.float32),
                        np.asarray(mean1, np.float32), np.asarray(var1, np.float32))
    s2, b2 = _bn_affine(np.asarray(gamma2, np.float32), np.asarray(beta2, np.float32),
                        np.asarray(mean2, np.float32), np.asarray(var2, np.float32))
    # conv PSUM holds 225*conv (15x-or-15a input, 15w weights) -> want 15*bn
    bnv = np.stack([s1 / 15.0, 15.0 * b1, s2 / 15.0, 15.0 * b2],
                   axis=1).astype(np.float32)  # [C, 4]

    w1r = _lhsT_taps(w1i).astype(np.float16)
    wdg = np.diag(np.float64(15.0) / s2).astype(np.float16)
    shared = {"w2p": w2p, "w1r": w1r, "bnv": bnv, "wdg": wdg}
    return [{"x15": np.ascontiguousarray(x15[i]), **shared}
            for i in range(N_CORES)]


def kernel(**inputs):
    from concourse.bass_utils import run_bass_kernel_spmd

    nc = _get_module()
    in_maps = _make_in_maps(**inputs)
    res = run_bass_kernel_spmd(nc, in_maps, core_ids=list(range(N_CORES)))
    _CACHE["last_res"] = res
    out = np.concatenate([np.asarray(r["out"], np.float32)
                          for r in res.results], axis=0)
    return out.reshape(B, C, H, W)
